# revision 39
# baseline (speedup 1.0000x reference)
"""DCRNN Trainium2 kernel: 8-way node sharding, 2-layer wavefront pipeline.

Decomposition:
- A row-normalized and A^2 computed on host; per-core operator column-slices
  P1 = [A^T[:,sh] | (A^2)^T[:,sh]] (computes [A@x | A^2@x] rows sh) and
  P2 = [A[:,sh] | A^2[:,sh]] (computes [A^T@x | (A^T)^2@x]) as fp8e4m3
  inputs scaled x1024 (entries ~5e-4 land mid-range; TRN e4m3 max +-240).
- xp = input (x) in_proj_w + b is rank-1 in the feature dim: the device
  diffuses the raw input (6 node-major columns, one per (b,t)) and the
  in_proj weights are folded into the l0 gate/cand x-projections on host
  (8-row folded weights; rows 5:8 carry the exact in_proj_b contribution).
- Activations feature-major per shard: state tiles [H=64, (b,n)=512] f32.
- Each diffused tensor gets a "bundle" [64, (b, op5, n256)] bf16 = identity
  + 4 operator applications via fp8 DoubleRow matmuls (2 contraction blocks
  per instruction): lhsT = AllGathered node-major activation (fp8 x16) in
  chunks [2,4,6,4], rhs = resident operator tiles; psum descaled 1/16384 on
  the bundle copies (split across DVE and ACT engines).
- Projections in bf16 (same PE rate as f32r, half the copy cost).
- 2-layer wavefront pipeline: layer-0 of step t+1 depends only on h0_t, so
  it runs concurrently with layer-1 of step t. 8 AllGather events total
  (fixed ~6-9us cost each dominates, so one full collective per round).
- Collective stalls are filled with real work: bundles + identity slots are
  allocated at AG-issue time so op-0 projections (and full h-parts when the
  bundle predates the AG) run during the transfer; filler matmuls keep the
  PE clock from HAM down-throttle (idle >3.4us halves the PE clock).
- In wave_b the l0 cand finishes first so its GRU + marshal overlap the l1
  cand matmuls. First AllGather is floored by the CC-init barrier (~58-79us,
  run-to-run jitter) + ~11us first-collective delay.

Hardware constraints honored (probed on trn2):
- every instruction <= 1 sync wait -> bacc.Bacc + nc.compile()
- DMA engines: sync/scalar/gpsimd only; gpsimd reserved for CC triggers
- DVE ops need 32-aligned partition bases; DMA partition APs must be real
  partition strides (bounce through DRAM to scatter psum rows)
- fp8 DoubleRow: lhsT pair-dim byte step %16 == 0 (li padded to 16 cols);
  fp8 PE transpose unsupported (marshal transposes stay bf16)
- collectives: internal DRAM tiles only, contiguous APs, serial CC stream
- DMA cannot read PSUM; transposes bounce PSUM -> DVE copy -> SBUF
"""
import numpy as np
import ml_dtypes
import concourse.bass as bass
import concourse.bacc as bacc
import concourse.tile as tile
from concourse import mybir
from concourse.bass_utils import run_bass_kernel_spmd

F32 = mybir.dt.float32
F32R = mybir.dt.float32r
BF16 = mybir.dt.bfloat16
F8 = mybir.dt.float8e4
DR = mybir.MatmulPerfMode.DoubleRow
AF = mybir.ActivationFunctionType

SOP, SACT = 1024.0, 16.0       # fp8 scales: operators, activations
DESC = 1.0 / (SOP * SACT)      # psum descale on bundle copies

N, H, B, SEQ, L = 2048, 64, 2, 3, 2
W = 8            # cores
NS = N // W      # 256 nodes per shard
KT = N // 128    # 16 contraction blocks
BN = B * NS      # 512 = (b, n) free size
RG = [list(range(W))]
PHASES = [("enc", 0), ("enc", 1), ("dec", 0), ("dec", 1)]  # dram row order
HB_BUFS, RHB_BUFS = 4, 2
FILL_1T, FILL_2T = 20, 28   # PE-warm fillers per 1/2-tensor AG round


def build_program():
    nc = bacc.Bacc(None, num_devices=W, name="dcrnn")

    # ---- DRAM inputs (per core) ----
    p1_in = nc.dram_tensor("p1_in", [N, 2 * NS], F8, kind="ExternalInput")
    p2_in = nc.dram_tensor("p2_in", [N, 2 * NS], F8, kind="ExternalInput")
    inp_nm_in = nc.dram_tensor("inp_nm", [N, 16], F8, kind="ExternalInput")
    xin_in = nc.dram_tensor("xin_in", [B * SEQ, NS], BF16, kind="ExternalInput")
    dstat_in = nc.dram_tensor("dstat_in", [3, B * NS], BF16, kind="ExternalInput")
    wfg_in = nc.dram_tensor("wfg_in", [8, 2 * H], BF16, kind="ExternalInput")
    wfc_in = nc.dram_tensor("wfc_in", [8, H], BF16, kind="ExternalInput")
    wg_in = nc.dram_tensor("wg_in", [4, 5 * 128, 2 * H], BF16, kind="ExternalInput")
    wc_in = nc.dram_tensor("wc_in", [4, 5 * 128, H], BF16, kind="ExternalInput")
    bg_in = nc.dram_tensor("bg_in", [4 * 2 * H, 1], F32, kind="ExternalInput")
    bc_in = nc.dram_tensor("bc_in", [4 * H, 1], F32, kind="ExternalInput")
    wout_in = nc.dram_tensor("wout_in", [H, 1], BF16, kind="ExternalInput")
    bout_in = nc.dram_tensor("bout_in", [1, 1], F32, kind="ExternalInput")
    ident_in = nc.dram_tensor("ident_in", [128, 128], F32, kind="ExternalInput")
    out_t = nc.dram_tensor("out", [1, BN], F32, kind="ExternalOutput")

    with tile.TileContext(nc) as tc:
        with (
            tc.tile_pool(name="persist", bufs=1) as persist,
            tc.tile_pool(name="lhstp", bufs=3) as lhstp,
            tc.tile_pool(name="hbp", bufs=HB_BUFS) as hbp,
            tc.tile_pool(name="rhbp", bufs=RHB_BUFS) as rhbp,
            tc.tile_pool(name="statep", bufs=2) as statep,
            tc.tile_pool(name="hstp", bufs=5) as hstp,
            tc.tile_pool(name="smallp", bufs=2) as smallp,
            tc.tile_pool(name="stgp", bufs=2) as stgp,
            tc.tile_pool(name="pdiff", bufs=4, space="PSUM") as pdiff,
            tc.tile_pool(name="pproj", bufs=2, space="PSUM") as pproj,
            tc.tile_pool(name="ptr", bufs=2, space="PSUM") as ptr,
            tc.tile_pool(name="dml", bufs=4, space="DRAM") as dml,
            tc.tile_pool(name="dms", bufs=4, space="DRAM") as dms,
        ):
            uid = [0]

            def nm(pfx):
                uid[0] += 1
                return f"{pfx}{uid[0]}"

            data_dma = [nc.sync, nc.scalar]

            def dma_eng(i):
                return data_dma[i % 2]

            # =================================================================
            # persistent SBUF
            # =================================================================
            ident = persist.tile([128, 128], F32, name="ident")
            nc.sync.dma_start(ident, ident_in.ap())
            ident_f8 = persist.tile([128, 128], F8, name="ident_f8")
            nc.vector.tensor_copy(ident_f8, ident)
            ident_bf = persist.tile([128, 128], BF16, name="ident_bf")
            nc.vector.tensor_copy(ident_bf, ident)

            # input node-major lhsT [128, kt, 16], fp8 x16; cols 6:16
            # zero-padded for DoubleRow 16B step alignment. Loaded first so
            # the d_in diffusion starts as early as possible.
            li = persist.tile([128, KT, 16], F8, name="li")
            nc.sync.dma_start(
                li, inp_nm_in.ap().rearrange("(k p) c -> p k c", p=128))

            # operator pairs, fp8 x1024
            rp1 = persist.tile([128, KT, 2 * NS], F8, name="rp1")
            rp2 = persist.tile([128, KT, 2 * NS], F8, name="rp2")
            for ck in range(4):
                rows = slice(ck * 4 * 128, (ck + 1) * 4 * 128)
                dma_eng(ck).dma_start(
                    rp1[:, ck * 4:(ck + 1) * 4, :],
                    p1_in.ap()[rows, :].rearrange("(k p) c -> p k c", p=128))
                dma_eng(ck + 1).dma_start(
                    rp2[:, ck * 4:(ck + 1) * 4, :],
                    p2_in.ap()[rows, :].rearrange("(k p) c -> p k c", p=128))

            # weights per phase key (f32r), split x-part / h-part
            wgx, wgh, wcx, wch, bgr_sb, bgu_sb, bc_sb = {}, {}, {}, {}, {}, {}, {}
            for pi, key in enumerate(PHASES):
                src_g = wg_in.ap()[pi, :, :].rearrange(
                    "(o p) u -> p o u", p=128)
                if key in (("enc", 1), ("dec", 1)):
                    wgx[key] = persist.tile([H, 5, 2 * H], BF16, name=f"wgx{pi}")
                    nc.sync.dma_start(wgx[key], src_g[0:H])
                wgh[key] = persist.tile([H, 5, 2 * H], BF16, name=f"wgh{pi}")
                nc.scalar.dma_start(wgh[key], src_g[H:2 * H])
                src_c = wc_in.ap()[pi, :, :].rearrange(
                    "(o p) u -> p o u", p=128)
                if key in (("enc", 1), ("dec", 1)):
                    wcx[key] = persist.tile([H, 5, H], BF16, name=f"wcx{pi}")
                    nc.sync.dma_start(wcx[key], src_c[0:H])
                wch[key] = persist.tile([H, 5, H], BF16, name=f"wch{pi}")
                nc.scalar.dma_start(wch[key], src_c[H:2 * H])
                bgr_sb[key] = persist.tile([H, 1], F32, name=f"bgr{pi}")
                nc.sync.dma_start(bgr_sb[key], bg_in.ap()[pi * 128: pi * 128 + H, :])
                bgu_sb[key] = persist.tile([H, 1], F32, name=f"bgu{pi}")
                nc.sync.dma_start(bgu_sb[key], bg_in.ap()[pi * 128 + H: pi * 128 + 2 * H, :])
                bc_sb[key] = persist.tile([H, 1], F32, name=f"bc{pi}")
                nc.sync.dma_start(bc_sb[key], bc_in.ap()[pi * H: (pi + 1) * H, :])
            wfg = persist.tile([8, 2 * H], BF16, name="wfg")
            nc.sync.dma_start(wfg, wfg_in.ap())
            wfc = persist.tile([8, H], BF16, name="wfc")
            nc.sync.dma_start(wfc, wfc_in.ap())
            wout_sb = persist.tile([H, 1], BF16, name="wout_sb")
            nc.sync.dma_start(wout_sb, wout_in.ap())
            bout_sb = persist.tile([1, 1], F32, name="bout_sb")
            nc.sync.dma_start(bout_sb, bout_in.ap())

            def fillers(n):
                # PE-warm junk matmuls into a recycled diffusion psum slot;
                # no cross-engine deps, keep HAM up during collective waits
                pf = pdiff.tile([128, 512], F32, name=nm("pf"), tag="pdiff")
                for i in range(n):
                    nc.tensor.matmul(pf, ident_f8, rp1[:, i % KT, :],
                                     start=True, stop=True)

            # =================================================================
            # helpers
            # =================================================================
            bundles = {}
            alloc_count = {"hb": 0, "rhb": 0}
            state = {}

            def bundle_alloc(name, pool, tag):
                t = pool.tile([H, B, 5, NS], BF16, name=nm("bun_" + name), tag=tag)
                alloc_count[tag] += 1
                bundles[name] = (t, alloc_count[tag], tag)
                return t

            def bundle_get(name):
                t, idx, tag = bundles[name]
                bufs = {"hb": HB_BUFS, "rhb": RHB_BUFS}[tag]
                assert idx > alloc_count[tag] - bufs, \
                    f"bundle {name} slot recycled ({idx} vs {alloc_count[tag]})"
                return t

            ag_pending = []   # list of (ag_out_half0, ag_out_half1, names, pools)

            def marshal_tensor(stg, ti, sname):
                """Cast one state tile to scaled fp8 node-major into stg."""
                t = state[sname]
                tb = smallp.tile([H, BN], BF16, name=nm("tb"), tag="tb")
                nc.vector.tensor_scalar_mul(tb, t, SACT)
                for b in range(B):
                    for nh in range(2):
                        pt = ptr.tile([128, H], BF16, name=nm("agt"), tag="ptr")
                        nc.tensor.transpose(
                            pt,
                            tb[:, b * NS + nh * 128: b * NS + (nh + 1) * 128],
                            ident_bf[0:H, 0:H])
                        nc.vector.tensor_copy(
                            stg[:, nh, ti * 128 + b * H: ti * 128 + (b + 1) * H],
                            pt)

            def ag_finalize(stg, Cg):
                ag_in = dml.tile([NS, Cg], F8, name=nm("agi"), tag="agin")
                nc.sync.dma_start(
                    ag_in.rearrange("(nh p) c -> p nh c", p=128), stg)
                ag_out = dms.tile([N, Cg], F8, name=nm("ago"),
                                  tag="agout", addr_space="Shared")
                nc.gpsimd.collective_compute(
                    "AllGather", mybir.AluOpType.bypass, replica_groups=RG,
                    ins=[ag_in.opt()], outs=[ag_out.opt()])
                return ag_out

            def ag_issue(names):
                Cg = 128 * len(names)
                stg = stgp.tile([128, 2, Cg], F8, name=nm("stg"), tag="stg")
                for ti, sname in enumerate(names):
                    marshal_tensor(stg, ti, sname)
                return ag_finalize(stg, Cg)

            def diffuse_pre(names, pool_tags):
                """At AG-issue time: alloc bundles + psums, write identity
                slots (local states) so op-0 projections can run in the
                collective stall."""
                buns, ps = [], []
                for ti, name in enumerate(names):
                    pool, tag = pool_tags[ti]
                    buns.append(bundle_alloc(name, pool, tag))
                    p1 = pdiff.tile([128, 512], F32, name=nm("p1"), tag="pdiff")
                    p2 = pdiff.tile([128, 512], F32, name=nm("p2"), tag="pdiff")
                    ps.append((p1, p2))
                    nc.vector.tensor_copy(
                        buns[ti][:, :, 0, :],
                        state[name].rearrange("p (b n) -> p b n", b=B))
                return (names, buns, ps)

            def diffuse_read(handle, outs):
                names, buns, ps = handle
                Cm = len(names)
                CHUNKS = [(0, 2), (2, 4), (6, 6), (12, 4)]
                for ck, (k0, klen) in enumerate(CHUNKS):
                    lt = lhstp.tile([128, klen, Cm * 128], F8,
                                    name=nm("lt"), tag=f"lt{ck}")
                    dma_eng(ck).dma_start(
                        lt,
                        outs[k0 * 128:(k0 + klen) * 128, :]
                        .rearrange("(k p) c -> p k c", p=128))
                    for k2p in range(klen // 2):   # DoubleRow: 2 blocks per mm
                        jj = k0 + 2 * k2p
                        for ti in range(Cm):
                            p1, p2 = ps[ti]
                            lts = lt[:, 2 * k2p:2 * k2p + 2,
                                     ti * 128:(ti + 1) * 128]
                            flags = dict(
                                start=(ck == 0 and k2p == 0),
                                stop=(ck == 3 and k2p == klen // 2 - 1))
                            nc.tensor.matmul(p1, lts, rp1[:, jj:jj + 2, :],
                                             perf_mode=DR, **flags)
                            nc.tensor.matmul(p2, lts, rp2[:, jj:jj + 2, :],
                                             perf_mode=DR, **flags)
                for ti, name in enumerate(names):
                    p1, p2 = ps[ti]
                    bun = buns[ti]
                    for b in range(B):
                        nc.vector.tensor_scalar_mul(bun[:, b, 1:3, :],
                                                    p1[b * H:(b + 1) * H, :],
                                                    DESC)
                        nc.scalar.activation(bun[:, b, 3:5, :],
                                             p2[b * H:(b + 1) * H, :],
                                             AF.Copy, scale=DESC)

            def proj_ops(psum, wt, bname, ops, start, stop):
                bun = bundle_get(bname)
                for i, op in enumerate(ops):
                    nc.tensor.matmul(psum, wt[:, op, :], bun[:, :, op, :],
                                     start=(start and i == 0),
                                     stop=(stop and i == len(ops) - 1))

            def proj_state(psum, wt, bname, start, stop):
                proj_ops(psum, wt, bname, [0, 1, 2, 3, 4], start, stop)

            def proj_fold(psum, wf, dx, start, stop):
                nc.tensor.matmul(psum, wf, dx, start=start, stop=stop)

            def gates_act(key, gps, need_r=True):
                r_t = None
                if need_r:
                    r_t = statep.tile([H, BN], F32, name=nm("r"), tag="r")
                    nc.scalar.activation(r_t, gps[0:H, :], AF.Sigmoid,
                                         bias=bgr_sb[key])
                u_t = statep.tile([H, BN], F32, name=nm("u"), tag="u")
                nc.scalar.activation(u_t, gps[H:2 * H, :], AF.Sigmoid,
                                     bias=bgu_sb[key])
                return r_t, u_t

            def rh_mul(r_t, h_name, cid):
                rh_t = statep.tile([H, BN], F32, name=nm("rh"), tag="rh" + cid)
                nc.vector.tensor_mul(rh_t, r_t, state[h_name])
                sname = f"rh_{cid}"
                state[sname] = rh_t
                return sname

            def gru_finish(key, cps, u_t, h_name, out_name):
                cand_t = statep.tile([H, BN], F32, name=nm("cand"), tag="cand")
                nc.scalar.activation(cand_t, cps, AF.Tanh, bias=bc_sb[key])
                hn = hstp.tile([H, BN], F32, name=nm("h"), tag="hst")
                tmp = statep.tile([H, BN], F32, name=nm("tmp"), tag="tmp")
                if h_name is None:
                    nc.vector.tensor_mul(tmp, u_t, cand_t)
                    nc.vector.tensor_sub(hn, cand_t, tmp)          # (1-u)*c
                else:
                    tmp2 = statep.tile([H, BN], F32, name=nm("tmp2"), tag="tmp2")
                    nc.vector.tensor_sub(tmp, state[h_name], cand_t)
                    nc.vector.tensor_mul(tmp2, u_t, tmp)
                    nc.vector.tensor_add(hn, cand_t, tmp2)         # c + u*(h-c)
                state[out_name] = hn
                return hn

            # =================================================================
            # setup: input diffusion (6 cols) + dx tiles
            # =================================================================
            p1d = pdiff.tile([B * SEQ, 512], F32, name="p1d", tag="pdiff")
            p2d = pdiff.tile([B * SEQ, 512], F32, name="p2d", tag="pdiff")
            for j2 in range(KT // 2):
                nc.tensor.matmul(p1d, li[:, 2 * j2:2 * j2 + 2, 0:B * SEQ],
                                 rp1[:, 2 * j2:2 * j2 + 2, :], perf_mode=DR,
                                 start=(j2 == 0), stop=(j2 == KT // 2 - 1))
                nc.tensor.matmul(p2d, li[:, 2 * j2:2 * j2 + 2, 0:B * SEQ],
                                 rp2[:, 2 * j2:2 * j2 + 2, :], perf_mode=DR,
                                 start=(j2 == 0), stop=(j2 == KT // 2 - 1))
            scr1 = smallp.tile([B * SEQ, 512], BF16, name="scr1", tag="scr")
            scr2 = smallp.tile([B * SEQ, 512], BF16, name="scr2", tag="scr")
            nc.vector.tensor_scalar_mul(scr1, p1d, DESC)
            nc.vector.tensor_scalar_mul(scr2, p2d, DESC)
            scrd = dml.tile([B * SEQ, 4 * NS], BF16, name="scrd", tag="scrd")
            nc.sync.dma_start(scrd[:, 0:2 * NS], scr1)
            nc.scalar.dma_start(scrd[:, 2 * NS:4 * NS], scr2)
            dx = []
            for t in range(SEQ):
                dxt = persist.tile([8, B, NS], BF16, name=f"dx{t}")
                for b in range(B):
                    c = b * SEQ + t
                    dma_eng(b).dma_start(dxt[0:1, b, :],
                                         xin_in.ap()[c:c + 1, :])
                    dma_eng(b + 1).dma_start(
                        dxt[1:5, b, :],
                        scrd[c:c + 1, :].rearrange("r (o n) -> (r o) n", o=4))
                nc.sync.dma_start(
                    dxt[5:8, :, :],
                    dstat_in.ap().rearrange("s (b n) -> s b n", b=B))
                dx.append(dxt)

            EG, CG = ("enc", 0), ("enc", 0)

            # =================================================================
            # W1: E0_1  (x = xp_0, h = 0)
            # =================================================================
            gps = pproj.tile([2 * H, BN], F32, name=nm("gps"), tag="pproj")
            proj_fold(gps, wfg, dx[0], True, True)
            _, u_t = gates_act(EG, gps, need_r=False)
            cps = pproj.tile([H, BN], F32, name=nm("cps"), tag="pproj")
            proj_fold(cps, wfc, dx[0], True, True)
            gru_finish(EG, cps, u_t, None, "h0_1")
            ag1 = ag_issue(["h0_1"])
            K11 = ("enc", 1)
            dh1 = diffuse_pre(["h0_1"], [(hbp, "hb")])
            # pre-emit during AG1: op-0 projections off the identity slot
            gps11 = pproj.tile([2 * H, BN], F32, name=nm("gps"), tag="pproj")
            proj_ops(gps11, wgx[K11], "h0_1", [0], True, False)
            gps02 = pproj.tile([2 * H, BN], F32, name=nm("gps"), tag="pproj")
            proj_fold(gps02, wfg, dx[1], True, False)
            proj_ops(gps02, wgh[EG], "h0_1", [0], False, False)
            fillers(FILL_1T)

            # =================================================================
            # W2a: diff(h0_1); E1_1 full; E0_2 gates -> AG[rh02, h1_1]
            # =================================================================
            diffuse_read(dh1, ag1)
            proj_ops(gps11, wgx[K11], "h0_1", [1, 2, 3, 4], False, True)
            proj_ops(gps02, wgh[EG], "h0_1", [1, 2, 3, 4], False, True)
            _, u11 = gates_act(K11, gps11, need_r=False)
            cps11 = pproj.tile([H, BN], F32, name=nm("cps"), tag="pproj")
            proj_state(cps11, wcx[K11], "h0_1", True, True)
            gru_finish(K11, cps11, u11, None, "h1_1")
            r02, u02 = gates_act(EG, gps02)
            rh02 = rh_mul(r02, "h0_1", "02")
            ag2 = ag_issue([rh02, "h1_1"])
            drh2 = diffuse_pre([rh02, "h1_1"], [(rhbp, "rhb"), (hbp, "hb")])
            # during AG2: E0_2 cand x-part + rh02 op-0
            cps02 = pproj.tile([H, BN], F32, name=nm("cps"), tag="pproj")
            proj_fold(cps02, wfc, dx[1], True, False)
            proj_ops(cps02, wch[CG], rh02, [0], False, False)
            fillers(FILL_2T)

            # =================================================================
            # W2b: diff(rh02, h1_1); E0_2 cand -> h0_2 -> AG[h0_2]
            # =================================================================
            diffuse_read(drh2, ag2)
            proj_ops(cps02, wch[CG], rh02, [1, 2, 3, 4], False, True)
            gru_finish(CG, cps02, u02, "h0_1", "h0_2")
            ag3 = ag_issue(["h0_2"])
            dh2 = diffuse_pre(["h0_2"], [(hbp, "hb")])
            # during AG3: E1_2 gates h-part is FULLY available (B(h1_1)
            # diffused in W2b) + op-0s off h0_2 identity
            g1 = pproj.tile([2 * H, BN], F32, name=nm("gps"), tag="pproj")
            proj_ops(g1, wgx[K11], "h0_2", [0], True, False)
            proj_state(g1, wgh[K11], "h1_1", False, False)
            g0 = pproj.tile([2 * H, BN], F32, name=nm("gps"), tag="pproj")
            proj_fold(g0, wfg, dx[2], True, False)
            proj_ops(g0, wgh[EG], "h0_2", [0], False, False)
            fillers(FILL_1T)

            # ===== steady-state wavefronts =====
            def wave_a(dh, ag_st, st_names, l1key, l0key, x1name, h1name,
                       l0_dx, h0name, cid1, cid0, g1, g0, h1_full):
                """Finish state diffusion + gates; issue rh AG; pre-emit
                cand partials + rh bundle setup during the stall."""
                diffuse_read(dh, ag_st)
                if h1_full:
                    proj_ops(g1, wgx[l1key], x1name, [1, 2, 3, 4], False, True)
                else:
                    proj_ops(g1, wgx[l1key], x1name, [1, 2, 3, 4], False, False)
                    proj_ops(g1, wgh[l1key], h1name, [1, 2, 3, 4], False, True)
                proj_ops(g0, wgh[l0key], h0name, [1, 2, 3, 4], False, True)
                r1_t = statep.tile([H, BN], F32, name=nm("r"), tag="r")
                nc.scalar.activation(r1_t, g1[0:H, :], AF.Sigmoid,
                                     bias=bgr_sb[l1key])
                r0_t = statep.tile([H, BN], F32, name=nm("r"), tag="r0")
                nc.scalar.activation(r0_t, g0[0:H, :], AF.Sigmoid,
                                     bias=bgr_sb[l0key])
                rh1 = rh_mul(r1_t, h1name, cid1)
                rh0 = rh_mul(r0_t, h0name, cid0)
                ag_rh = ag_issue([rh1, rh0])
                drh = diffuse_pre([rh1, rh0], [(rhbp, "rhb"), (rhbp, "rhb")])
                # during the rh AG: cand x-parts (bundles complete) + rh op-0
                c1 = pproj.tile([H, BN], F32, name=nm("cps"), tag="pproj")
                proj_state(c1, wcx[l1key], x1name, True, False)
                proj_ops(c1, wch[l1key], rh1, [0], False, False)
                c0 = pproj.tile([H, BN], F32, name=nm("cps"), tag="pproj")
                if l0_dx is not None:
                    proj_fold(c0, wfc, l0_dx, True, False)
                proj_ops(c0, wch[l0key], rh0, [0], l0_dx is None, False)
                u1 = statep.tile([H, BN], F32, name=nm("u"), tag="u")
                nc.scalar.activation(u1, g1[H:2 * H, :], AF.Sigmoid,
                                     bias=bgu_sb[l1key])
                u0 = statep.tile([H, BN], F32, name=nm("u"), tag="u0")
                nc.scalar.activation(u0, g0[H:2 * H, :], AF.Sigmoid,
                                     bias=bgu_sb[l0key])
                fillers(FILL_2T)
                return drh, ag_rh, rh1, rh0, u1, u0, c1, c0

            def wave_b(drh, ag_rh, rh1, rh0, u1, u0, c1, c0, l1key, l0key,
                       h1name, h0name, out1, out0, nxt):
                """Finish rh diffusion + cands + GRU; issue state AG; pre-emit
                next wave's gate op-0s during the stall. nxt is None (last
                wave) or (nl1key, nl0key, nx1, nh1, nh0, ndx, nh1_full)."""
                diffuse_read(drh, ag_rh)
                # l0 cand first: its GRU + marshal run on scalar/vector while
                # the PE still chews the l1 cand matmuls
                proj_ops(c0, wch[l0key], rh0, [1, 2, 3, 4], False, True)
                gru_finish(l0key, c0, u0, h0name, out0)
                stg = stgp.tile([128, 2, 256], F8, name=nm("stg"), tag="stg")
                proj_ops(c1, wch[l1key], rh1, [1, 2, 3, 4], False, True)
                marshal_tensor(stg, 0, out0)
                gru_finish(l1key, c1, u1, h1name, out1)
                marshal_tensor(stg, 1, out1)
                ag_st = ag_finalize(stg, 256)
                dh = diffuse_pre([out0, out1], [(hbp, "hb"), (hbp, "hb")])
                g1n = g0n = None
                if nxt is not None:
                    nl1, nl0, nx1, nh1, nh0, ndx, nh1f = nxt
                    g1n = pproj.tile([2 * H, BN], F32, name=nm("gps"),
                                     tag="pproj")
                    proj_ops(g1n, wgx[nl1], nx1, [0], True, False)
                    if nh1f:
                        proj_state(g1n, wgh[nl1], nh1, False, False)
                    else:
                        proj_ops(g1n, wgh[nl1], nh1, [0], False, False)
                    if nl0 is not None:
                        g0n = pproj.tile([2 * H, BN], F32, name=nm("gps"),
                                         tag="pproj")
                        if ndx is not None:
                            proj_fold(g0n, wfg, ndx, True, False)
                        proj_ops(g0n, wgh[nl0], nh0, [0], ndx is None, False)
                fillers(FILL_2T)
                return dh, ag_st, g1n, g0n

            # W3: E1_2 (x=h0_2, h=h1_1), E0_3 (x=xp_2, h=h0_2)
            KE1, KE0, KD0, KD = ("enc", 1), ("enc", 0), ("dec", 0), ("dec", 1)
            drh, ag_rh, rh1, rh0, u1, u0, c1, c0 = wave_a(
                dh2, ag3, ["h0_2"], KE1, KE0, "h0_2", "h1_1",
                dx[2], "h0_2", "12", "03", g1, g0, True)
            dh, ag5, g1, g0 = wave_b(
                drh, ag_rh, rh1, rh0, u1, u0, c1, c0, KE1, KE0,
                "h1_1", "h0_2", "h1_2", "h0_3",
                (KE1, KD0, "h0_3", "h1_2", "h0_3", None, False))

            # W4: E1_3 (x=h0_3, h=h1_2), D0 (x=None, h=h0_3)
            drh, ag_rh, rh1, rh0, u1, u0, c1, c0 = wave_a(
                dh, ag5, ["h0_3", "h1_2"], KE1, KD0, "h0_3", "h1_2",
                None, "h0_3", "13", "0d", g1, g0, False)
            dh, ag7, g1, g0 = wave_b(
                drh, ag_rh, rh1, rh0, u1, u0, c1, c0, KE1, KD0,
                "h1_2", "h0_3", "h1_3", "hd0",
                (KD, None, "hd0", "h1_3", None, None, False))

            # =================================================================
            # W5: D1 (x=hd0, h=h1_3) -> output
            # =================================================================
            diffuse_read(dh, ag7)
            proj_ops(g1, wgx[KD], "hd0", [1, 2, 3, 4], False, False)
            proj_ops(g1, wgh[KD], "h1_3", [1, 2, 3, 4], False, True)
            r1_t = statep.tile([H, BN], F32, name=nm("r"), tag="r")
            nc.scalar.activation(r1_t, g1[0:H, :], AF.Sigmoid,
                                 bias=bgr_sb[KD])
            rh1d = rh_mul(r1_t, "h1_3", "1d")
            ag8 = ag_issue([rh1d])
            drh = diffuse_pre([rh1d], [(rhbp, "rhb")])
            c1 = pproj.tile([H, BN], F32, name=nm("cps"), tag="pproj")
            proj_state(c1, wcx[KD], "hd0", True, False)
            proj_ops(c1, wch[KD], rh1d, [0], False, False)
            u1 = statep.tile([H, BN], F32, name=nm("u"), tag="u")
            nc.scalar.activation(u1, g1[H:2 * H, :], AF.Sigmoid,
                                 bias=bgu_sb[KD])
            fillers(FILL_1T)
            diffuse_read(drh, ag8)
            proj_ops(c1, wch[KD], rh1d, [1, 2, 3, 4], False, True)
            h1d = gru_finish(KD, c1, u1, "h1_3", "h1_d")

            h1d_bf = smallp.tile([H, BN], BF16, name="h1d_bf", tag="tb")
            nc.vector.tensor_copy(h1d_bf, h1d)
            ops = pproj.tile([1, BN], F32, name="ops", tag="pproj")
            nc.tensor.matmul(ops, wout_sb, h1d_bf, start=True, stop=True)
            out_sb = smallp.tile([1, BN], F32, name="out_sb", tag="outsb")
            nc.vector.tensor_scalar_add(out_sb, ops, bout_sb)
            nc.sync.dma_start(out_t.ap(), out_sb)

    nc.compile()
    return nc


def make_in_maps(inputs):
    adj = np.asarray(inputs["adj"], np.float64)
    A = adj + np.eye(N) * 1e-6
    A = (A / (A.sum(axis=1, keepdims=True) + 1e-8)).astype(np.float32)
    A2 = (A @ A).astype(np.float32)
    inp = np.asarray(inputs["inputs"], np.float32)          # (B, SEQ, N)
    w_in = np.asarray(inputs["in_proj_w"], np.float32)[0]   # (H,)
    b_in = np.asarray(inputs["in_proj_b"], np.float32)      # (H,)

    # node-major input, cols c = b*SEQ + t; fp8 x16, processing order
    inp_nm = np.zeros((N, 16), np.float32)
    inp_nm[:, 0:B * SEQ] = inp.transpose(2, 0, 1).reshape(N, B * SEQ) * SACT
    inp_nm = np.ascontiguousarray(inp_nm).astype(ml_dtypes.float8_e4m3)

    # static bias-diffusion vectors: ones, A^T@1, (A^T)^2@1
    sAT = A.sum(axis=0).astype(np.float32)
    sAT2 = A2.sum(axis=0).astype(np.float32)

    wg = np.ascontiguousarray(np.concatenate(
        [np.asarray(inputs["enc_gate_w"], np.float32),
         np.asarray(inputs["dec_gate_w"], np.float32)], axis=0))
    wg_bf = wg.astype(ml_dtypes.bfloat16)
    wc = np.ascontiguousarray(np.concatenate(
        [np.asarray(inputs["enc_cand_w"], np.float32),
         np.asarray(inputs["dec_cand_w"], np.float32)], axis=0))
    wc_bf = wc.astype(ml_dtypes.bfloat16)
    bg = np.ascontiguousarray(np.concatenate(
        [np.asarray(inputs["enc_gate_b"], np.float32),
         np.asarray(inputs["dec_gate_b"], np.float32)], axis=0).reshape(4 * 2 * H, 1))
    bc = np.ascontiguousarray(np.concatenate(
        [np.asarray(inputs["enc_cand_b"], np.float32),
         np.asarray(inputs["dec_cand_b"], np.float32)], axis=0).reshape(4 * H, 1))

    # fold in_proj into enc-l0 x-projections: rows 0-4 = sum_h w[h]*W[op,h,:],
    # rows 5-7 = bias folds pairing with [ones, sAT, sAT2]
    def fold(Wm, width):
        Wo = Wm.reshape(5, 128, width)[:, 0:H, :]   # x-part rows
        wf = np.zeros((8, width), np.float32)
        wf[0:5] = np.einsum('h,ohu->ou', w_in, Wo)
        bb = np.einsum('h,ohu->ou', b_in, Wo)       # (5, width)
        wf[5] = bb[0] + bb[1] + bb[2]
        wf[6] = bb[3]
        wf[7] = bb[4]
        return wf
    wfg = fold(wg[0], 2 * H)
    wfc = fold(wc[0], H)

    wout = np.ascontiguousarray(np.asarray(inputs["out_proj_w"], np.float32))
    bout = np.asarray(inputs["out_proj_b"], np.float32).reshape(1, 1)
    ident = np.eye(128, dtype=np.float32)

    in_maps = []
    for r in range(W):
        sh = slice(r * NS, (r + 1) * NS)
        p1 = np.concatenate([A.T[:, sh], A2.T[:, sh]], axis=1) * SOP
        p2 = np.concatenate([A[:, sh], A2[:, sh]], axis=1) * SOP
        xin = np.ascontiguousarray(
            inp[:, :, sh].reshape(B * SEQ, NS)).astype(ml_dtypes.bfloat16)
        dstat = np.ascontiguousarray(np.broadcast_to(
            np.stack([np.ones(NS, np.float32), sAT[sh], sAT2[sh]])[:, None, :],
            (3, B, NS)).reshape(3, B * NS)).astype(ml_dtypes.bfloat16)
        in_maps.append({
            "p1_in": np.ascontiguousarray(p1).astype(ml_dtypes.float8_e4m3),
            "p2_in": np.ascontiguousarray(p2).astype(ml_dtypes.float8_e4m3),
            "inp_nm": inp_nm,
            "xin_in": xin,
            "dstat_in": dstat,
            "wfg_in": wfg.astype(ml_dtypes.bfloat16),
            "wfc_in": wfc.astype(ml_dtypes.bfloat16),
            "wg_in": wg_bf, "wc_in": wc_bf, "bg_in": bg, "bc_in": bc,
            "wout_in": wout.astype(ml_dtypes.bfloat16), "bout_in": bout, "ident_in": ident,
        })
    return in_maps


def assemble_output(results):
    out = np.zeros((B, 1, N), np.float32)
    for r in range(W):
        res = results[r]["out"]  # [1, BN]
        for b in range(B):
            out[b, 0, r * NS:(r + 1) * NS] = res[0, b * NS:(b + 1) * NS]
    return out


_CACHE = {}


def get_program():
    if "nc" not in _CACHE:
        _CACHE["nc"] = build_program()
    return _CACHE["nc"]


def kernel(**inputs):
    nc = get_program()
    in_maps = make_in_maps(inputs)
    res = run_bass_kernel_spmd(nc, in_maps, core_ids=list(range(W)))
    return assemble_output(res.results)


# revision 40
# speedup vs baseline: 1.0219x; 1.0219x over previous
"""DCRNN Trainium2 kernel: 8-way node sharding, 2-layer wavefront pipeline.

Decomposition:
- A row-normalized and A^2 computed on host; per-core operator column-slices
  P1 = [A^T[:,sh] | (A^2)^T[:,sh]] (computes [A@x | A^2@x] rows sh) and
  P2 = [A[:,sh] | A^2[:,sh]] (computes [A^T@x | (A^T)^2@x]) as fp8e4m3
  inputs scaled x1024 (entries ~5e-4 land mid-range; TRN e4m3 max +-240).
- xp = input (x) in_proj_w + b is rank-1 in the feature dim: the device
  diffuses the raw input (6 node-major columns, one per (b,t)) and the
  in_proj weights are folded into the l0 gate/cand x-projections on host
  (8-row folded weights; rows 5:8 carry the exact in_proj_b contribution).
- Activations feature-major per shard: state tiles [H=64, (b,n)=512] f32.
- Each diffused tensor gets a "bundle" [64, (b, op5, n256)] bf16 = identity
  + 4 operator applications via fp8 DoubleRow matmuls (2 contraction blocks
  per instruction): lhsT = AllGathered node-major activation (fp8 x16) in
  chunks [2,4,6,4], rhs = resident operator tiles; psum descaled 1/16384 on
  the bundle copies (split across DVE and ACT engines).
- Projections in bf16 (same PE rate as f32r, half the copy cost).
- 2-layer wavefront pipeline: layer-0 of step t+1 depends only on h0_t, so
  it runs concurrently with layer-1 of step t. 8 AllGather events total
  (fixed ~6-9us cost each dominates, so one full collective per round).
- Collective stalls are filled with real work: bundles + identity slots are
  allocated at AG-issue time so op-0 projections (and full h-parts when the
  bundle predates the AG) run during the transfer; filler matmuls keep the
  PE clock from HAM down-throttle (idle >3.4us halves the PE clock).
- In wave_b the l0 cand finishes first so its GRU + marshal overlap the l1
  cand matmuls. First AllGather is floored by the CC-init barrier (~58-79us,
  run-to-run jitter) + ~11us first-collective delay.

Hardware constraints honored (probed on trn2):
- every instruction <= 1 sync wait -> bacc.Bacc + nc.compile()
- DMA engines: sync/scalar/gpsimd only; gpsimd reserved for CC triggers
- DVE ops need 32-aligned partition bases; DMA partition APs must be real
  partition strides (bounce through DRAM to scatter psum rows)
- fp8 DoubleRow: lhsT pair-dim byte step %16 == 0 (li padded to 16 cols);
  fp8 PE transpose unsupported (marshal transposes stay bf16)
- collectives: internal DRAM tiles only, contiguous APs, serial CC stream
- DMA cannot read PSUM; transposes bounce PSUM -> DVE copy -> SBUF
"""
import numpy as np
import ml_dtypes
import concourse.bass as bass
import concourse.bacc as bacc
import concourse.tile as tile
from concourse import mybir
from concourse.bass_utils import run_bass_kernel_spmd

F32 = mybir.dt.float32
F32R = mybir.dt.float32r
BF16 = mybir.dt.bfloat16
F8 = mybir.dt.float8e4
DR = mybir.MatmulPerfMode.DoubleRow
AF = mybir.ActivationFunctionType

SOP, SACT = 1024.0, 16.0       # fp8 scales: operators, activations
DESC = 1.0 / (SOP * SACT)      # psum descale on bundle copies

N, H, B, SEQ, L = 2048, 64, 2, 3, 2
W = 8            # cores
NS = N // W      # 256 nodes per shard
KT = N // 128    # 16 contraction blocks
BN = B * NS      # 512 = (b, n) free size
RG = [list(range(W))]
PHASES = [("enc", 0), ("enc", 1), ("dec", 0), ("dec", 1)]  # dram row order
HB_BUFS, RHB_BUFS = 4, 2
FILL_1T, FILL_2T = 18, 24   # PE-warm fillers per 1/2-tensor AG round


def build_program():
    nc = bacc.Bacc(None, num_devices=W, name="dcrnn")

    # ---- DRAM inputs (per core) ----
    p1_in = nc.dram_tensor("p1_in", [N, 2 * NS], F8, kind="ExternalInput")
    p2_in = nc.dram_tensor("p2_in", [N, 2 * NS], F8, kind="ExternalInput")
    inp_nm_in = nc.dram_tensor("inp_nm", [N, 16], F8, kind="ExternalInput")
    xin_in = nc.dram_tensor("xin_in", [B * SEQ, NS], BF16, kind="ExternalInput")
    dstat_in = nc.dram_tensor("dstat_in", [3, B * NS], BF16, kind="ExternalInput")
    wfg_in = nc.dram_tensor("wfg_in", [8, 2 * H], BF16, kind="ExternalInput")
    wfc_in = nc.dram_tensor("wfc_in", [8, H], BF16, kind="ExternalInput")
    wg_in = nc.dram_tensor("wg_in", [4, 5 * 128, 2 * H], BF16, kind="ExternalInput")
    wc_in = nc.dram_tensor("wc_in", [4, 5 * 128, H], BF16, kind="ExternalInput")
    bg_in = nc.dram_tensor("bg_in", [4 * 2 * H, 1], F32, kind="ExternalInput")
    bc_in = nc.dram_tensor("bc_in", [4 * H, 1], F32, kind="ExternalInput")
    wout_in = nc.dram_tensor("wout_in", [H, 1], BF16, kind="ExternalInput")
    bout_in = nc.dram_tensor("bout_in", [1, 1], F32, kind="ExternalInput")
    ident_in = nc.dram_tensor("ident_in", [128, 128], F32, kind="ExternalInput")
    out_t = nc.dram_tensor("out", [1, BN], F32, kind="ExternalOutput")

    with tile.TileContext(nc) as tc:
        with (
            tc.tile_pool(name="persist", bufs=1) as persist,
            tc.tile_pool(name="lhstp", bufs=3) as lhstp,
            tc.tile_pool(name="hbp", bufs=HB_BUFS) as hbp,
            tc.tile_pool(name="rhbp", bufs=RHB_BUFS) as rhbp,
            tc.tile_pool(name="statep", bufs=2) as statep,
            tc.tile_pool(name="hstp", bufs=5) as hstp,
            tc.tile_pool(name="smallp", bufs=2) as smallp,
            tc.tile_pool(name="stgp", bufs=2) as stgp,
            tc.tile_pool(name="pdiff", bufs=4, space="PSUM") as pdiff,
            tc.tile_pool(name="pproj", bufs=2, space="PSUM") as pproj,
            tc.tile_pool(name="ptr", bufs=2, space="PSUM") as ptr,
            tc.tile_pool(name="dml", bufs=4, space="DRAM") as dml,
            tc.tile_pool(name="dms", bufs=4, space="DRAM") as dms,
        ):
            uid = [0]

            def nm(pfx):
                uid[0] += 1
                return f"{pfx}{uid[0]}"

            data_dma = [nc.sync, nc.scalar]

            def dma_eng(i):
                return data_dma[i % 2]

            # =================================================================
            # persistent SBUF
            # =================================================================
            ident = persist.tile([128, 128], F32, name="ident")
            nc.sync.dma_start(ident, ident_in.ap())
            ident_f8 = persist.tile([128, 128], F8, name="ident_f8")
            nc.vector.tensor_copy(ident_f8, ident)
            ident_bf = persist.tile([128, 128], BF16, name="ident_bf")
            nc.vector.tensor_copy(ident_bf, ident)

            # input node-major lhsT [128, kt, 16], fp8 x16; cols 6:16
            # zero-padded for DoubleRow 16B step alignment. Loaded first so
            # the d_in diffusion starts as early as possible.
            li = persist.tile([128, KT, 16], F8, name="li")
            nc.sync.dma_start(
                li, inp_nm_in.ap().rearrange("(k p) c -> p k c", p=128))

            # operator pairs, fp8 x1024
            rp1 = persist.tile([128, KT, 2 * NS], F8, name="rp1")
            rp2 = persist.tile([128, KT, 2 * NS], F8, name="rp2")
            for ck in range(4):
                rows = slice(ck * 4 * 128, (ck + 1) * 4 * 128)
                dma_eng(ck).dma_start(
                    rp1[:, ck * 4:(ck + 1) * 4, :],
                    p1_in.ap()[rows, :].rearrange("(k p) c -> p k c", p=128))
                dma_eng(ck + 1).dma_start(
                    rp2[:, ck * 4:(ck + 1) * 4, :],
                    p2_in.ap()[rows, :].rearrange("(k p) c -> p k c", p=128))

            # weights per phase key (f32r), split x-part / h-part
            wgx, wgh, wcx, wch, bgr_sb, bgu_sb, bc_sb = {}, {}, {}, {}, {}, {}, {}
            for pi, key in enumerate(PHASES):
                src_g = wg_in.ap()[pi, :, :].rearrange(
                    "(o p) u -> p o u", p=128)
                if key in (("enc", 1), ("dec", 1)):
                    wgx[key] = persist.tile([H, 5, 2 * H], BF16, name=f"wgx{pi}")
                    nc.sync.dma_start(wgx[key], src_g[0:H])
                wgh[key] = persist.tile([H, 5, 2 * H], BF16, name=f"wgh{pi}")
                nc.scalar.dma_start(wgh[key], src_g[H:2 * H])
                src_c = wc_in.ap()[pi, :, :].rearrange(
                    "(o p) u -> p o u", p=128)
                if key in (("enc", 1), ("dec", 1)):
                    wcx[key] = persist.tile([H, 5, H], BF16, name=f"wcx{pi}")
                    nc.sync.dma_start(wcx[key], src_c[0:H])
                wch[key] = persist.tile([H, 5, H], BF16, name=f"wch{pi}")
                nc.scalar.dma_start(wch[key], src_c[H:2 * H])
                bgr_sb[key] = persist.tile([H, 1], F32, name=f"bgr{pi}")
                nc.sync.dma_start(bgr_sb[key], bg_in.ap()[pi * 128: pi * 128 + H, :])
                bgu_sb[key] = persist.tile([H, 1], F32, name=f"bgu{pi}")
                nc.sync.dma_start(bgu_sb[key], bg_in.ap()[pi * 128 + H: pi * 128 + 2 * H, :])
                bc_sb[key] = persist.tile([H, 1], F32, name=f"bc{pi}")
                nc.sync.dma_start(bc_sb[key], bc_in.ap()[pi * H: (pi + 1) * H, :])
            wfg = persist.tile([8, 2 * H], BF16, name="wfg")
            nc.sync.dma_start(wfg, wfg_in.ap())
            wfc = persist.tile([8, H], BF16, name="wfc")
            nc.sync.dma_start(wfc, wfc_in.ap())
            wout_sb = persist.tile([H, 1], BF16, name="wout_sb")
            nc.sync.dma_start(wout_sb, wout_in.ap())
            bout_sb = persist.tile([1, 1], F32, name="bout_sb")
            nc.sync.dma_start(bout_sb, bout_in.ap())

            def fillers(n):
                # PE-warm junk matmuls into a recycled diffusion psum slot;
                # no cross-engine deps, keep HAM up during collective waits
                pf = pdiff.tile([128, 512], F32, name=nm("pf"), tag="pdiff")
                for i in range(n):
                    nc.tensor.matmul(pf, ident_f8, rp1[:, i % KT, :],
                                     start=True, stop=True)

            # =================================================================
            # helpers
            # =================================================================
            bundles = {}
            alloc_count = {"hb": 0, "rhb": 0}
            state = {}

            def bundle_alloc(name, pool, tag):
                t = pool.tile([H, B, 5, NS], BF16, name=nm("bun_" + name), tag=tag)
                alloc_count[tag] += 1
                bundles[name] = (t, alloc_count[tag], tag)
                return t

            def bundle_get(name):
                t, idx, tag = bundles[name]
                bufs = {"hb": HB_BUFS, "rhb": RHB_BUFS}[tag]
                assert idx > alloc_count[tag] - bufs, \
                    f"bundle {name} slot recycled ({idx} vs {alloc_count[tag]})"
                return t

            ag_pending = []   # list of (ag_out_half0, ag_out_half1, names, pools)

            def marshal_tensor(stg, ti, sname):
                """Cast one state tile to scaled fp8 node-major into stg."""
                t = state[sname]
                tb = smallp.tile([H, BN], BF16, name=nm("tb"), tag="tb")
                nc.vector.tensor_scalar_mul(tb, t, SACT)
                for b in range(B):
                    for nh in range(2):
                        pt = ptr.tile([128, H], BF16, name=nm("agt"), tag="ptr")
                        nc.tensor.transpose(
                            pt,
                            tb[:, b * NS + nh * 128: b * NS + (nh + 1) * 128],
                            ident_bf[0:H, 0:H])
                        nc.vector.tensor_copy(
                            stg[:, nh, ti * 128 + b * H: ti * 128 + (b + 1) * H],
                            pt)

            def ag_finalize(stg, Cg):
                ag_in = dml.tile([NS, Cg], F8, name=nm("agi"), tag="agin")
                nc.sync.dma_start(
                    ag_in.rearrange("(nh p) c -> p nh c", p=128), stg)
                ag_out = dms.tile([N, Cg], F8, name=nm("ago"),
                                  tag="agout", addr_space="Shared")
                nc.gpsimd.collective_compute(
                    "AllGather", mybir.AluOpType.bypass, replica_groups=RG,
                    ins=[ag_in.opt()], outs=[ag_out.opt()])
                return ag_out

            def ag_issue(names):
                Cg = 128 * len(names)
                stg = stgp.tile([128, 2, Cg], F8, name=nm("stg"), tag="stg")
                for ti, sname in enumerate(names):
                    marshal_tensor(stg, ti, sname)
                return ag_finalize(stg, Cg)

            def diffuse_pre(names, pool_tags):
                """At AG-issue time: alloc bundles + psums, write identity
                slots (local states) so op-0 projections can run in the
                collective stall."""
                buns, ps = [], []
                for ti, name in enumerate(names):
                    pool, tag = pool_tags[ti]
                    buns.append(bundle_alloc(name, pool, tag))
                    p1 = pdiff.tile([128, 512], F32, name=nm("p1"), tag="pdiff")
                    p2 = pdiff.tile([128, 512], F32, name=nm("p2"), tag="pdiff")
                    ps.append((p1, p2))
                    nc.vector.tensor_copy(
                        buns[ti][:, :, 0, :],
                        state[name].rearrange("p (b n) -> p b n", b=B))
                return (names, buns, ps)

            def diffuse_read(handle, outs):
                names, buns, ps = handle
                Cm = len(names)
                CHUNKS = [(0, 2), (2, 4), (6, 6), (12, 4)]
                for ck, (k0, klen) in enumerate(CHUNKS):
                    lt = lhstp.tile([128, klen, Cm * 128], F8,
                                    name=nm("lt"), tag=f"lt{ck}")
                    dma_eng(ck).dma_start(
                        lt,
                        outs[k0 * 128:(k0 + klen) * 128, :]
                        .rearrange("(k p) c -> p k c", p=128))
                    for k2p in range(klen // 2):   # DoubleRow: 2 blocks per mm
                        jj = k0 + 2 * k2p
                        for ti in range(Cm):
                            p1, p2 = ps[ti]
                            lts = lt[:, 2 * k2p:2 * k2p + 2,
                                     ti * 128:(ti + 1) * 128]
                            flags = dict(
                                start=(ck == 0 and k2p == 0),
                                stop=(ck == 3 and k2p == klen // 2 - 1))
                            nc.tensor.matmul(p1, lts, rp1[:, jj:jj + 2, :],
                                             perf_mode=DR, **flags)
                            nc.tensor.matmul(p2, lts, rp2[:, jj:jj + 2, :],
                                             perf_mode=DR, **flags)
                for ti, name in enumerate(names):
                    p1, p2 = ps[ti]
                    bun = buns[ti]
                    for b in range(B):
                        nc.vector.tensor_scalar_mul(bun[:, b, 1:3, :],
                                                    p1[b * H:(b + 1) * H, :],
                                                    DESC)
                        nc.scalar.activation(bun[:, b, 3:5, :],
                                             p2[b * H:(b + 1) * H, :],
                                             AF.Copy, scale=DESC)

            def proj_ops(psum, wt, bname, ops, start, stop):
                bun = bundle_get(bname)
                for i, op in enumerate(ops):
                    nc.tensor.matmul(psum, wt[:, op, :], bun[:, :, op, :],
                                     start=(start and i == 0),
                                     stop=(stop and i == len(ops) - 1))

            def proj_state(psum, wt, bname, start, stop):
                proj_ops(psum, wt, bname, [0, 1, 2, 3, 4], start, stop)

            def proj_fold(psum, wf, dx, start, stop):
                nc.tensor.matmul(psum, wf, dx, start=start, stop=stop)

            def gates_act(key, gps, need_r=True):
                r_t = None
                if need_r:
                    r_t = statep.tile([H, BN], F32, name=nm("r"), tag="r")
                    nc.scalar.activation(r_t, gps[0:H, :], AF.Sigmoid,
                                         bias=bgr_sb[key])
                u_t = statep.tile([H, BN], F32, name=nm("u"), tag="u")
                nc.scalar.activation(u_t, gps[H:2 * H, :], AF.Sigmoid,
                                     bias=bgu_sb[key])
                return r_t, u_t

            def rh_mul(r_t, h_name, cid):
                rh_t = statep.tile([H, BN], F32, name=nm("rh"), tag="rh" + cid)
                nc.vector.tensor_mul(rh_t, r_t, state[h_name])
                sname = f"rh_{cid}"
                state[sname] = rh_t
                return sname

            def gru_finish(key, cps, u_t, h_name, out_name):
                cand_t = statep.tile([H, BN], F32, name=nm("cand"), tag="cand")
                nc.scalar.activation(cand_t, cps, AF.Tanh, bias=bc_sb[key])
                hn = hstp.tile([H, BN], F32, name=nm("h"), tag="hst")
                tmp = statep.tile([H, BN], F32, name=nm("tmp"), tag="tmp")
                if h_name is None:
                    nc.vector.tensor_mul(tmp, u_t, cand_t)
                    nc.vector.tensor_sub(hn, cand_t, tmp)          # (1-u)*c
                else:
                    tmp2 = statep.tile([H, BN], F32, name=nm("tmp2"), tag="tmp2")
                    nc.vector.tensor_sub(tmp, state[h_name], cand_t)
                    nc.vector.tensor_mul(tmp2, u_t, tmp)
                    nc.vector.tensor_add(hn, cand_t, tmp2)         # c + u*(h-c)
                state[out_name] = hn
                return hn

            # =================================================================
            # setup: input diffusion (6 cols) + dx tiles
            # =================================================================
            p1d = pdiff.tile([B * SEQ, 512], F32, name="p1d", tag="pdiff")
            p2d = pdiff.tile([B * SEQ, 512], F32, name="p2d", tag="pdiff")
            for j2 in range(KT // 2):
                nc.tensor.matmul(p1d, li[:, 2 * j2:2 * j2 + 2, 0:B * SEQ],
                                 rp1[:, 2 * j2:2 * j2 + 2, :], perf_mode=DR,
                                 start=(j2 == 0), stop=(j2 == KT // 2 - 1))
                nc.tensor.matmul(p2d, li[:, 2 * j2:2 * j2 + 2, 0:B * SEQ],
                                 rp2[:, 2 * j2:2 * j2 + 2, :], perf_mode=DR,
                                 start=(j2 == 0), stop=(j2 == KT // 2 - 1))
            scr1 = smallp.tile([B * SEQ, 512], BF16, name="scr1", tag="scr")
            scr2 = smallp.tile([B * SEQ, 512], BF16, name="scr2", tag="scr")
            nc.vector.tensor_scalar_mul(scr1, p1d, DESC)
            nc.vector.tensor_scalar_mul(scr2, p2d, DESC)
            scrd = dml.tile([B * SEQ, 4 * NS], BF16, name="scrd", tag="scrd")
            nc.sync.dma_start(scrd[:, 0:2 * NS], scr1)
            nc.scalar.dma_start(scrd[:, 2 * NS:4 * NS], scr2)
            dx = []
            for t in range(SEQ):
                dxt = persist.tile([8, B, NS], BF16, name=f"dx{t}")
                for b in range(B):
                    c = b * SEQ + t
                    dma_eng(b).dma_start(dxt[0:1, b, :],
                                         xin_in.ap()[c:c + 1, :])
                    dma_eng(b + 1).dma_start(
                        dxt[1:5, b, :],
                        scrd[c:c + 1, :].rearrange("r (o n) -> (r o) n", o=4))
                nc.sync.dma_start(
                    dxt[5:8, :, :],
                    dstat_in.ap().rearrange("s (b n) -> s b n", b=B))
                dx.append(dxt)

            EG, CG = ("enc", 0), ("enc", 0)

            # =================================================================
            # W1: E0_1  (x = xp_0, h = 0)
            # =================================================================
            gps = pproj.tile([2 * H, BN], F32, name=nm("gps"), tag="pproj")
            proj_fold(gps, wfg, dx[0], True, True)
            _, u_t = gates_act(EG, gps, need_r=False)
            cps = pproj.tile([H, BN], F32, name=nm("cps"), tag="pproj")
            proj_fold(cps, wfc, dx[0], True, True)
            gru_finish(EG, cps, u_t, None, "h0_1")
            ag1 = ag_issue(["h0_1"])
            K11 = ("enc", 1)
            dh1 = diffuse_pre(["h0_1"], [(hbp, "hb")])
            # pre-emit during AG1: op-0 projections off the identity slot
            gps11 = pproj.tile([2 * H, BN], F32, name=nm("gps"), tag="pproj")
            proj_ops(gps11, wgx[K11], "h0_1", [0], True, False)
            gps02 = pproj.tile([2 * H, BN], F32, name=nm("gps"), tag="pproj")
            proj_fold(gps02, wfg, dx[1], True, False)
            proj_ops(gps02, wgh[EG], "h0_1", [0], False, False)
            fillers(FILL_1T)

            # =================================================================
            # W2a: diff(h0_1); E1_1 full; E0_2 gates -> AG[rh02, h1_1]
            # =================================================================
            diffuse_read(dh1, ag1)
            proj_ops(gps11, wgx[K11], "h0_1", [1, 2, 3, 4], False, True)
            proj_ops(gps02, wgh[EG], "h0_1", [1, 2, 3, 4], False, True)
            _, u11 = gates_act(K11, gps11, need_r=False)
            cps11 = pproj.tile([H, BN], F32, name=nm("cps"), tag="pproj")
            proj_state(cps11, wcx[K11], "h0_1", True, True)
            gru_finish(K11, cps11, u11, None, "h1_1")
            r02, u02 = gates_act(EG, gps02)
            rh02 = rh_mul(r02, "h0_1", "02")
            ag2 = ag_issue([rh02, "h1_1"])
            drh2 = diffuse_pre([rh02, "h1_1"], [(rhbp, "rhb"), (hbp, "hb")])
            # during AG2: E0_2 cand x-part + rh02 op-0
            cps02 = pproj.tile([H, BN], F32, name=nm("cps"), tag="pproj")
            proj_fold(cps02, wfc, dx[1], True, False)
            proj_ops(cps02, wch[CG], rh02, [0], False, False)
            fillers(FILL_2T)

            # =================================================================
            # W2b: diff(rh02, h1_1); E0_2 cand -> h0_2 -> AG[h0_2]
            # =================================================================
            diffuse_read(drh2, ag2)
            proj_ops(cps02, wch[CG], rh02, [1, 2, 3, 4], False, True)
            gru_finish(CG, cps02, u02, "h0_1", "h0_2")
            ag3 = ag_issue(["h0_2"])
            dh2 = diffuse_pre(["h0_2"], [(hbp, "hb")])
            # during AG3: E1_2 gates h-part is FULLY available (B(h1_1)
            # diffused in W2b) + op-0s off h0_2 identity
            g1 = pproj.tile([2 * H, BN], F32, name=nm("gps"), tag="pproj")
            proj_ops(g1, wgx[K11], "h0_2", [0], True, False)
            proj_state(g1, wgh[K11], "h1_1", False, False)
            g0 = pproj.tile([2 * H, BN], F32, name=nm("gps"), tag="pproj")
            proj_fold(g0, wfg, dx[2], True, False)
            proj_ops(g0, wgh[EG], "h0_2", [0], False, False)
            fillers(FILL_1T)

            # ===== steady-state wavefronts =====
            def wave_a(dh, ag_st, st_names, l1key, l0key, x1name, h1name,
                       l0_dx, h0name, cid1, cid0, g1, g0, h1_full):
                """Finish state diffusion + gates; issue rh AG; pre-emit
                cand partials + rh bundle setup during the stall."""
                diffuse_read(dh, ag_st)
                if h1_full:
                    proj_ops(g1, wgx[l1key], x1name, [1, 2, 3, 4], False, True)
                else:
                    proj_ops(g1, wgx[l1key], x1name, [1, 2, 3, 4], False, False)
                    proj_ops(g1, wgh[l1key], h1name, [1, 2, 3, 4], False, True)
                proj_ops(g0, wgh[l0key], h0name, [1, 2, 3, 4], False, True)
                r1_t = statep.tile([H, BN], F32, name=nm("r"), tag="r")
                nc.scalar.activation(r1_t, g1[0:H, :], AF.Sigmoid,
                                     bias=bgr_sb[l1key])
                r0_t = statep.tile([H, BN], F32, name=nm("r"), tag="r0")
                nc.scalar.activation(r0_t, g0[0:H, :], AF.Sigmoid,
                                     bias=bgr_sb[l0key])
                rh1 = rh_mul(r1_t, h1name, cid1)
                rh0 = rh_mul(r0_t, h0name, cid0)
                ag_rh = ag_issue([rh1, rh0])
                drh = diffuse_pre([rh1, rh0], [(rhbp, "rhb"), (rhbp, "rhb")])
                # during the rh AG: cand x-parts (bundles complete) + rh op-0
                c1 = pproj.tile([H, BN], F32, name=nm("cps"), tag="pproj")
                proj_state(c1, wcx[l1key], x1name, True, False)
                proj_ops(c1, wch[l1key], rh1, [0], False, False)
                c0 = pproj.tile([H, BN], F32, name=nm("cps"), tag="pproj")
                if l0_dx is not None:
                    proj_fold(c0, wfc, l0_dx, True, False)
                proj_ops(c0, wch[l0key], rh0, [0], l0_dx is None, False)
                u1 = statep.tile([H, BN], F32, name=nm("u"), tag="u")
                nc.scalar.activation(u1, g1[H:2 * H, :], AF.Sigmoid,
                                     bias=bgu_sb[l1key])
                u0 = statep.tile([H, BN], F32, name=nm("u"), tag="u0")
                nc.scalar.activation(u0, g0[H:2 * H, :], AF.Sigmoid,
                                     bias=bgu_sb[l0key])
                fillers(FILL_2T)
                return drh, ag_rh, rh1, rh0, u1, u0, c1, c0

            def wave_b(drh, ag_rh, rh1, rh0, u1, u0, c1, c0, l1key, l0key,
                       h1name, h0name, out1, out0, nxt):
                """Finish rh diffusion + cands + GRU; issue state AG; pre-emit
                next wave's gate op-0s during the stall. nxt is None (last
                wave) or (nl1key, nl0key, nx1, nh1, nh0, ndx, nh1_full)."""
                diffuse_read(drh, ag_rh)
                # l0 cand first: its GRU + marshal run on scalar/vector while
                # the PE still chews the l1 cand matmuls
                proj_ops(c0, wch[l0key], rh0, [1, 2, 3, 4], False, True)
                gru_finish(l0key, c0, u0, h0name, out0)
                stg = stgp.tile([128, 2, 256], F8, name=nm("stg"), tag="stg")
                proj_ops(c1, wch[l1key], rh1, [1, 2, 3, 4], False, True)
                marshal_tensor(stg, 0, out0)
                gru_finish(l1key, c1, u1, h1name, out1)
                marshal_tensor(stg, 1, out1)
                ag_st = ag_finalize(stg, 256)
                dh = diffuse_pre([out0, out1], [(hbp, "hb"), (hbp, "hb")])
                g1n = g0n = None
                if nxt is not None:
                    nl1, nl0, nx1, nh1, nh0, ndx, nh1f = nxt
                    g1n = pproj.tile([2 * H, BN], F32, name=nm("gps"),
                                     tag="pproj")
                    proj_ops(g1n, wgx[nl1], nx1, [0], True, False)
                    if nh1f:
                        proj_state(g1n, wgh[nl1], nh1, False, False)
                    else:
                        proj_ops(g1n, wgh[nl1], nh1, [0], False, False)
                    if nl0 is not None:
                        g0n = pproj.tile([2 * H, BN], F32, name=nm("gps"),
                                         tag="pproj")
                        if ndx is not None:
                            proj_fold(g0n, wfg, ndx, True, False)
                        proj_ops(g0n, wgh[nl0], nh0, [0], ndx is None, False)
                fillers(FILL_2T)
                return dh, ag_st, g1n, g0n

            # W3: E1_2 (x=h0_2, h=h1_1), E0_3 (x=xp_2, h=h0_2)
            KE1, KE0, KD0, KD = ("enc", 1), ("enc", 0), ("dec", 0), ("dec", 1)
            drh, ag_rh, rh1, rh0, u1, u0, c1, c0 = wave_a(
                dh2, ag3, ["h0_2"], KE1, KE0, "h0_2", "h1_1",
                dx[2], "h0_2", "12", "03", g1, g0, True)
            dh, ag5, g1, g0 = wave_b(
                drh, ag_rh, rh1, rh0, u1, u0, c1, c0, KE1, KE0,
                "h1_1", "h0_2", "h1_2", "h0_3",
                (KE1, KD0, "h0_3", "h1_2", "h0_3", None, False))

            # W4: E1_3 (x=h0_3, h=h1_2), D0 (x=None, h=h0_3)
            drh, ag_rh, rh1, rh0, u1, u0, c1, c0 = wave_a(
                dh, ag5, ["h0_3", "h1_2"], KE1, KD0, "h0_3", "h1_2",
                None, "h0_3", "13", "0d", g1, g0, False)
            dh, ag7, g1, g0 = wave_b(
                drh, ag_rh, rh1, rh0, u1, u0, c1, c0, KE1, KD0,
                "h1_2", "h0_3", "h1_3", "hd0",
                (KD, None, "hd0", "h1_3", None, None, False))

            # =================================================================
            # W5: D1 (x=hd0, h=h1_3) -> output
            # =================================================================
            diffuse_read(dh, ag7)
            proj_ops(g1, wgx[KD], "hd0", [1, 2, 3, 4], False, False)
            proj_ops(g1, wgh[KD], "h1_3", [1, 2, 3, 4], False, True)
            r1_t = statep.tile([H, BN], F32, name=nm("r"), tag="r")
            nc.scalar.activation(r1_t, g1[0:H, :], AF.Sigmoid,
                                 bias=bgr_sb[KD])
            rh1d = rh_mul(r1_t, "h1_3", "1d")
            ag8 = ag_issue([rh1d])
            drh = diffuse_pre([rh1d], [(rhbp, "rhb")])
            c1 = pproj.tile([H, BN], F32, name=nm("cps"), tag="pproj")
            proj_state(c1, wcx[KD], "hd0", True, False)
            proj_ops(c1, wch[KD], rh1d, [0], False, False)
            u1 = statep.tile([H, BN], F32, name=nm("u"), tag="u")
            nc.scalar.activation(u1, g1[H:2 * H, :], AF.Sigmoid,
                                 bias=bgu_sb[KD])
            fillers(FILL_1T)
            diffuse_read(drh, ag8)
            proj_ops(c1, wch[KD], rh1d, [1, 2, 3, 4], False, True)
            h1d = gru_finish(KD, c1, u1, "h1_3", "h1_d")

            h1d_bf = smallp.tile([H, BN], BF16, name="h1d_bf", tag="tb")
            nc.vector.tensor_copy(h1d_bf, h1d)
            ops = pproj.tile([1, BN], F32, name="ops", tag="pproj")
            nc.tensor.matmul(ops, wout_sb, h1d_bf, start=True, stop=True)
            out_sb = smallp.tile([1, BN], F32, name="out_sb", tag="outsb")
            nc.vector.tensor_scalar_add(out_sb, ops, bout_sb)
            nc.sync.dma_start(out_t.ap(), out_sb)

    nc.compile()
    return nc


def make_in_maps(inputs):
    adj = np.asarray(inputs["adj"], np.float64)
    A = adj + np.eye(N) * 1e-6
    A = (A / (A.sum(axis=1, keepdims=True) + 1e-8)).astype(np.float32)
    A2 = (A @ A).astype(np.float32)
    inp = np.asarray(inputs["inputs"], np.float32)          # (B, SEQ, N)
    w_in = np.asarray(inputs["in_proj_w"], np.float32)[0]   # (H,)
    b_in = np.asarray(inputs["in_proj_b"], np.float32)      # (H,)

    # node-major input, cols c = b*SEQ + t; fp8 x16, processing order
    inp_nm = np.zeros((N, 16), np.float32)
    inp_nm[:, 0:B * SEQ] = inp.transpose(2, 0, 1).reshape(N, B * SEQ) * SACT
    inp_nm = np.ascontiguousarray(inp_nm).astype(ml_dtypes.float8_e4m3)

    # static bias-diffusion vectors: ones, A^T@1, (A^T)^2@1
    sAT = A.sum(axis=0).astype(np.float32)
    sAT2 = A2.sum(axis=0).astype(np.float32)

    wg = np.ascontiguousarray(np.concatenate(
        [np.asarray(inputs["enc_gate_w"], np.float32),
         np.asarray(inputs["dec_gate_w"], np.float32)], axis=0))
    wg_bf = wg.astype(ml_dtypes.bfloat16)
    wc = np.ascontiguousarray(np.concatenate(
        [np.asarray(inputs["enc_cand_w"], np.float32),
         np.asarray(inputs["dec_cand_w"], np.float32)], axis=0))
    wc_bf = wc.astype(ml_dtypes.bfloat16)
    bg = np.ascontiguousarray(np.concatenate(
        [np.asarray(inputs["enc_gate_b"], np.float32),
         np.asarray(inputs["dec_gate_b"], np.float32)], axis=0).reshape(4 * 2 * H, 1))
    bc = np.ascontiguousarray(np.concatenate(
        [np.asarray(inputs["enc_cand_b"], np.float32),
         np.asarray(inputs["dec_cand_b"], np.float32)], axis=0).reshape(4 * H, 1))

    # fold in_proj into enc-l0 x-projections: rows 0-4 = sum_h w[h]*W[op,h,:],
    # rows 5-7 = bias folds pairing with [ones, sAT, sAT2]
    def fold(Wm, width):
        Wo = Wm.reshape(5, 128, width)[:, 0:H, :]   # x-part rows
        wf = np.zeros((8, width), np.float32)
        wf[0:5] = np.einsum('h,ohu->ou', w_in, Wo)
        bb = np.einsum('h,ohu->ou', b_in, Wo)       # (5, width)
        wf[5] = bb[0] + bb[1] + bb[2]
        wf[6] = bb[3]
        wf[7] = bb[4]
        return wf
    wfg = fold(wg[0], 2 * H)
    wfc = fold(wc[0], H)

    wout = np.ascontiguousarray(np.asarray(inputs["out_proj_w"], np.float32))
    bout = np.asarray(inputs["out_proj_b"], np.float32).reshape(1, 1)
    ident = np.eye(128, dtype=np.float32)

    in_maps = []
    for r in range(W):
        sh = slice(r * NS, (r + 1) * NS)
        p1 = np.concatenate([A.T[:, sh], A2.T[:, sh]], axis=1) * SOP
        p2 = np.concatenate([A[:, sh], A2[:, sh]], axis=1) * SOP
        xin = np.ascontiguousarray(
            inp[:, :, sh].reshape(B * SEQ, NS)).astype(ml_dtypes.bfloat16)
        dstat = np.ascontiguousarray(np.broadcast_to(
            np.stack([np.ones(NS, np.float32), sAT[sh], sAT2[sh]])[:, None, :],
            (3, B, NS)).reshape(3, B * NS)).astype(ml_dtypes.bfloat16)
        in_maps.append({
            "p1_in": np.ascontiguousarray(p1).astype(ml_dtypes.float8_e4m3),
            "p2_in": np.ascontiguousarray(p2).astype(ml_dtypes.float8_e4m3),
            "inp_nm": inp_nm,
            "xin_in": xin,
            "dstat_in": dstat,
            "wfg_in": wfg.astype(ml_dtypes.bfloat16),
            "wfc_in": wfc.astype(ml_dtypes.bfloat16),
            "wg_in": wg_bf, "wc_in": wc_bf, "bg_in": bg, "bc_in": bc,
            "wout_in": wout.astype(ml_dtypes.bfloat16), "bout_in": bout, "ident_in": ident,
        })
    return in_maps


def assemble_output(results):
    out = np.zeros((B, 1, N), np.float32)
    for r in range(W):
        res = results[r]["out"]  # [1, BN]
        for b in range(B):
            out[b, 0, r * NS:(r + 1) * NS] = res[0, b * NS:(b + 1) * NS]
    return out


_CACHE = {}


def get_program():
    if "nc" not in _CACHE:
        _CACHE["nc"] = build_program()
    return _CACHE["nc"]


def kernel(**inputs):
    nc = get_program()
    in_maps = make_in_maps(inputs)
    res = run_bass_kernel_spmd(nc, in_maps, core_ids=list(range(W)))
    return assemble_output(res.results)
